# revision 7
# baseline (speedup 1.0000x reference)
"""Multi-head attention (B=8, N=1024, C=1024, H=16) on 8 Trainium2 NeuronCores.

Sharding: pure data-parallel — one batch element per core, weights replicated,
no collectives.

Per-core algorithm (all matmuls fp32r on the PE at full rate):
  phase 1a: qk projection.  qkT[d, n] = wqkT[c, d].T @ xT[c, n], d in [0, 2048).
            Bounced through DRAM (SBUF pressure) for per-head streaming later.
  phase 1b: v projection into *natural* [m, d_v] layout, stored interleaved as
            [m, 16*(64+1)] with a ones-column appended per head (the ones
            column makes the PV matmul emit softmax denominators for free).
  phase 2:  per head pair (row-packed K=64 matmuls on disjoint partition
            halves):
              S^T[m, n] = kT.T @ qT          (keys on partitions)
              expS = exp(SCALE * S^T)        (ACT, straight out of PSUM)
              U_aug[65, n] = v_aug.T @ expS  (rows 0..63 = unnormalized PV,
                                              row 64 = softmax denominator)
            denominators: gather -> reciprocal_approx_fast -> DRAM ->
            partition-broadcast DMA -> fused normalize (tensor_mul) into the
            attention-output-transposed buffer attn_outT[c, n].
  phase 3:  out[n, d] = attn_outT[c, n].T @ wpT[c, d] + b  (bias via
            broadcast tile + tensor_add), DMA to DRAM.
"""

import sys

if "/opt/trn_rl_repo" not in sys.path:
    sys.path.insert(0, "/opt/trn_rl_repo")

from contextlib import ExitStack

import numpy as np

import concourse.bass as bass
import concourse.mybir as mybir
from concourse import bacc
import concourse.tile as tile
from concourse import bass_utils

B, N, C, H = 8, 1024, 1024, 16
HD = C // H          # 64
SCALE = HD ** -0.5   # 0.125
P = 128              # SBUF partitions
NT = 512             # moving-dim tile (fp32 PSUM bank limit)
NCH = C // P         # 8 contraction chunks over channels
NMT = N // P         # 8 token tiles of 128
NNT = N // NT        # 2 token tiles of 512
F32 = mybir.dt.float32
F32R = mybir.dt.float32r
EXP = mybir.ActivationFunctionType.Exp


def build_module():
    nc = bacc.Bacc("TRN2", target_bir_lowering=False, debug=False, num_devices=B)

    xT = nc.dram_tensor("xT", [C, N], F32R, kind="ExternalInput").ap()
    wqkT = nc.dram_tensor("wqkT", [C, 2 * C], F32R, kind="ExternalInput").ap()
    wvT = nc.dram_tensor("wvT", [C, C], F32R, kind="ExternalInput").ap()
    wpT = nc.dram_tensor("wpT", [C, C], F32R, kind="ExternalInput").ap()
    bias = nc.dram_tensor("bias_bc", [P, C], F32, kind="ExternalInput").ap()
    ones_col = nc.dram_tensor("ones_col", [P, H], F32R, kind="ExternalInput").ap()
    out = nc.dram_tensor("out", [N, C], F32, kind="ExternalOutput").ap()

    with tile.TileContext(nc) as tc, ExitStack() as ctx:
        dram = ctx.enter_context(tc.tile_pool(name="dram", bufs=1, space="DRAM"))
        qkT_d = dram.tile([2 * C, N], F32R, tag="qkT_d", name="qkT_d")
        rden_d = dram.tile([H * NNT, NT], F32, tag="rden_d", name="rden_d")

        # 8 slots of [128, 1024]: first holds xT, later recycled for expS^T.
        xt_pool = ctx.enter_context(tc.tile_pool(name="xt", bufs=8))
        wqk_pool = ctx.enter_context(tc.tile_pool(name="wqk", bufs=10))
        wst_pool = ctx.enter_context(tc.tile_pool(name="wst", bufs=10))
        stage_pool = ctx.enter_context(tc.tile_pool(name="stage", bufs=4))
        vsb_pool = ctx.enter_context(tc.tile_pool(name="vsb", bufs=1))
        aot_pool = ctx.enter_context(tc.tile_pool(name="aot", bufs=1))
        qkp_pool = ctx.enter_context(tc.tile_pool(name="qkp", bufs=4))
        usb_pool = ctx.enter_context(tc.tile_pool(name="usb", bufs=6))
        den_pool = ctx.enter_context(tc.tile_pool(name="den", bufs=2))
        rbc_pool = ctx.enter_context(tc.tile_pool(name="rbc", bufs=4))
        one_pool = ctx.enter_context(tc.tile_pool(name="one", bufs=1))
        psum = ctx.enter_context(tc.tile_pool(name="psum", bufs=4, space="PSUM"))
        psum_u = ctx.enter_context(tc.tile_pool(name="psum_u", bufs=2, space="PSUM"))

        # ---------- input loads ----------
        xts = []
        for t in range(NCH):
            xt_t = xt_pool.tile([P, N], F32R, tag="xt", name=f"xt{t}")
            nc.sync.dma_start(xt_t, xT[t * P : (t + 1) * P, :])
            xts.append(xt_t)
        bias_sb = one_pool.tile([P, C], F32, tag="bias", name="bias_sb")
        nc.sync.dma_start(bias_sb, bias)

        # ---------- phase 1a: qk projection -> DRAM bounce ----------
        for dg in range(8):  # 256-wide d-groups over the 2048 qk channels
            wq_tiles = []
            for ck in range(NCH):
                wq_t = wqk_pool.tile([P, 256], F32R, tag="wqk", name=f"wq{dg}_{ck}")
                nc.sync.dma_start(
                    wq_t, wqkT[ck * P : (ck + 1) * P, dg * 256 : (dg + 1) * 256]
                )
                wq_tiles.append(wq_t)
            for ds_ in range(2):
                dt = dg * 2 + ds_
                for nt in range(NNT):
                    ps_qk = psum.tile([P, NT], F32, tag="ps", name=f"psqk{dt}_{nt}")
                    for ck in range(NCH):
                        nc.tensor.matmul(
                            ps_qk,
                            lhsT=(wq_tiles[ck][:, ds_ * P : (ds_ + 1) * P]),
                            rhs=(xts[ck][:, nt * NT : (nt + 1) * NT]),
                            start=(ck == 0),
                            stop=(ck == NCH - 1),
                        )
                    st = stage_pool.tile([P, NT], F32R, tag="stage", name=f"st{dt}_{nt}")
                    nc.vector.tensor_copy(st, ps_qk)
                    nc.sync.dma_start(
                        qkT_d[dt * P : (dt + 1) * P, nt * NT : (nt + 1) * NT], st
                    )

        # ---------- phase 1b: v projection (natural layout + ones cols) ----------
        vsb = []
        for mt in range(NMT):
            v_t = vsb_pool.tile([P, H * (HD + 1)], F32R, tag=f"v{mt}", name=f"v{mt}")
            nc.sync.dma_start(
                v_t.rearrange("p (h w) -> p h w", w=HD + 1)[:, :, HD : HD + 1], ones_col
            )
            vsb.append(v_t)
        for dvt in range(NNT):  # two 512-wide halves of d_v (heads 8*dvt..8*dvt+7)
            wv_tiles = []
            for ck in range(NCH):
                wv_t = wst_pool.tile([P, NT], F32R, tag="wst", name=f"wv{dvt}_{ck}")
                nc.sync.dma_start(
                    wv_t, wvT[ck * P : (ck + 1) * P, dvt * NT : (dvt + 1) * NT]
                )
                wv_tiles.append(wv_t)
            for mt in range(NMT):
                ps_v = psum.tile([P, NT], F32, tag="ps", name=f"psv{mt}_{dvt}")
                for ck in range(NCH):
                    nc.tensor.matmul(
                        ps_v,
                        lhsT=(xts[ck][:, mt * P : (mt + 1) * P]),
                        rhs=(wv_tiles[ck]),
                        start=(ck == 0),
                        stop=(ck == NCH - 1),
                    )
                dst = vsb[mt].rearrange("p (h w) -> p h w", w=HD + 1)[
                    :, dvt * 8 : (dvt + 1) * 8, 0:HD
                ]
                nc.vector.tensor_copy(dst, ps_v.rearrange("p (h w) -> p h w", w=HD))

        # ---------- attention-output accumulator (attn_outT[c, n]) ----------
        aot = []
        for t in range(NCH):
            a_t = aot_pool.tile([P, N], F32R, tag=f"aot{t}", name=f"aot{t}")
            aot.append(a_t)

        # ---------- phase 2: attention, per head pair ----------
        for pair in range(H // 2):
            hA = 2 * pair
            qp = qkp_pool.tile([P, N], F32R, tag="qp", name=f"qp{pair}")
            nc.sync.dma_start(qp, qkT_d[hA * HD : hA * HD + P, :])
            kp = qkp_pool.tile([P, N], F32R, tag="kp", name=f"kp{pair}")
            nc.sync.dma_start(kp, qkT_d[C + hA * HD : C + hA * HD + P, :])

            pair_units = []
            for j in range(2):  # head within pair; partitions j*64..j*64+63
                h = hA + j
                pl = slice(j * HD, (j + 1) * HD)
                for nt in range(NNT):
                    exps = [
                        xt_pool.tile([P, N], F32R, tag="xt", name=f"e{h}_{nt}_{q}")
                        for q in range(4)
                    ]
                    for mc in range(NMT):
                        ps_s = psum.tile([P, NT], F32, tag="ps", name=f"pss{h}_{nt}_{mc}")
                        nc.tensor.matmul(
                            ps_s,
                            lhsT=(kp[pl, mc * P : (mc + 1) * P]),
                            rhs=(qp[pl, nt * NT : (nt + 1) * NT]),
                            start=True,
                            stop=True,
                        )
                        nc.scalar.activation(
                            exps[mc // 2][:, (mc % 2) * NT : (mc % 2 + 1) * NT],
                            ps_s,
                            EXP,
                            scale=SCALE,
                        )
                    ps_u = psum_u.tile([HD + 1, NT], F32, tag="pu", name=f"psu{h}_{nt}")
                    for mc in range(NMT):
                        nc.tensor.matmul(
                            ps_u,
                            lhsT=(vsb[mc][:, h * (HD + 1) : (h + 1) * (HD + 1)]),
                            rhs=(exps[mc // 2][:, (mc % 2) * NT : (mc % 2 + 1) * NT]),
                            start=(mc == 0),
                            stop=(mc == NMT - 1),
                        )
                    u_sb = usb_pool.tile([HD + 1, NT], F32, tag="usb", name=f"u{h}_{nt}")
                    nc.vector.tensor_copy(u_sb, ps_u)
                    pair_units.append((h, nt, u_sb))

            # denominators for the 4 (head, nt) units of this pair
            den_g = den_pool.tile([4, NT], F32, tag="den", name=f"den{pair}")
            for i, (h, nt, u_sb) in enumerate(pair_units):
                nc.sync.dma_start(den_g[i : i + 1, :], u_sb[HD : HD + 1, :])
            rden = den_pool.tile([4, NT], F32, tag="rden", name=f"rden{pair}")
            nc.vector.reciprocal_approx_fast(out=rden, in_=den_g)
            nc.sync.dma_start(rden_d[pair * 4 : pair * 4 + 4, :], rden)
            for i, (h, nt, u_sb) in enumerate(pair_units):
                rbc = rbc_pool.tile([HD, NT], F32, tag="rbc", name=f"rbc{h}_{nt}")
                src = rden_d[pair * 4 + i : pair * 4 + i + 1, :]
                bsrc = bass.AP(
                    tensor=src.tensor,
                    offset=src.offset,
                    ap=[[0, HD], list(src.ap[-1])],
                )
                nc.gpsimd.dma_start(out=rbc, in_=bsrc)
                ct, prow = h // 2, (h % 2) * HD
                nc.vector.tensor_mul(
                    aot[ct][prow : prow + HD, nt * NT : (nt + 1) * NT],
                    u_sb[0:HD, :],
                    rbc,
                )

        # ---------- phase 3: output projection + bias ----------
        for dt in range(NNT):
            wp_tiles = []
            for ck in range(NCH):
                wp_t = wst_pool.tile([P, NT], F32R, tag="wst", name=f"wp{dt}_{ck}")
                nc.sync.dma_start(
                    wp_t, wpT[ck * P : (ck + 1) * P, dt * NT : (dt + 1) * NT]
                )
                wp_tiles.append(wp_t)
            for nt2 in range(NMT):
                ps_o = psum.tile([P, NT], F32, tag="ps", name=f"pso{dt}_{nt2}")
                for ck in range(NCH):
                    nc.tensor.matmul(
                        ps_o,
                        lhsT=(aot[ck][:, nt2 * P : (nt2 + 1) * P]),
                        rhs=(wp_tiles[ck]),
                        start=(ck == 0),
                        stop=(ck == NCH - 1),
                    )
                o_sb = stage_pool.tile([P, NT], F32, tag="stage", name=f"o{dt}_{nt2}")
                nc.vector.tensor_add(o_sb, ps_o, bias_sb[:, dt * NT : (dt + 1) * NT])
                nc.sync.dma_start(
                    out[nt2 * P : (nt2 + 1) * P, dt * NT : (dt + 1) * NT], o_sb
                )

    nc.compile()
    return nc


def make_in_maps(x, w_qkv, w_proj, b_proj):
    wqkT = np.ascontiguousarray(w_qkv[: 2 * C].T)
    wvT = np.ascontiguousarray(w_qkv[2 * C :].T)
    wpT = np.ascontiguousarray(w_proj.T)
    bias_bc = np.ascontiguousarray(np.broadcast_to(b_proj, (P, C)))
    ones = np.ones((P, H), dtype=np.float32)
    in_maps = []
    for b in range(B):
        in_maps.append(
            {
                "xT": np.ascontiguousarray(x[b].T),
                "wqkT": wqkT,
                "wvT": wvT,
                "wpT": wpT,
                "bias_bc": bias_bc,
                "ones_col": ones,
            }
        )
    return in_maps


_CACHED_NC = None


def kernel(x, w_qkv, w_proj, b_proj):
    global _CACHED_NC
    x = np.asarray(x, dtype=np.float32)
    w_qkv = np.asarray(w_qkv, dtype=np.float32)
    w_proj = np.asarray(w_proj, dtype=np.float32)
    b_proj = np.asarray(b_proj, dtype=np.float32)
    if _CACHED_NC is None:
        _CACHED_NC = build_module()
    nc = _CACHED_NC
    in_maps = make_in_maps(x, w_qkv, w_proj, b_proj)
    res = bass_utils.run_bass_kernel_spmd(nc, in_maps, core_ids=list(range(B)))
    return np.stack([res.results[b]["out"] for b in range(B)], axis=0)


if __name__ == "__main__":
    nc = build_module()
    ninst = sum(len(b.instructions) for b in nc.m.functions[0].blocks)
    print("module built ok;", ninst, "instructions")


# revision 8
# speedup vs baseline: 1.3137x; 1.3137x over previous
"""Multi-head attention (B=8, N=1024, C=1024, H=16) on 8 Trainium2 NeuronCores.

Sharding: pure data-parallel — one batch element per core, weights replicated,
no collectives.

Per-core algorithm (all matmuls fp32r on the PE at full rate):
  phase 1a: qk projection.  qkT[d, n] = wqkT[c, d].T @ xT[c, n], d in [0, 2048).
            Bounced through DRAM (SBUF pressure) for per-head streaming later.
  phase 1b: v projection into *natural* [m, d_v] layout, stored interleaved as
            [m, 16*(64+1)] with a ones-column appended per head (the ones
            column makes the PV matmul emit softmax denominators for free).
  phase 2:  per head pair (row-packed K=64 matmuls on disjoint partition
            halves):
              S^T[m, n] = kT.T @ qT          (keys on partitions)
              expS = exp(SCALE * S^T)        (ACT, straight out of PSUM)
              U_aug[65, n] = v_aug.T @ expS  (rows 0..63 = unnormalized PV,
                                              row 64 = softmax denominator)
            denominators: gather -> reciprocal_approx_fast -> DRAM ->
            partition-broadcast DMA -> fused normalize (tensor_mul) into the
            attention-output-transposed buffer attn_outT[c, n].
  phase 3:  out[n, d] = attn_outT[c, n].T @ wpT[c, d] + b  (bias via
            broadcast tile + tensor_add), DMA to DRAM.
"""

import sys

if "/opt/trn_rl_repo" not in sys.path:
    sys.path.insert(0, "/opt/trn_rl_repo")

from contextlib import ExitStack

import numpy as np

import concourse.bass as bass
import concourse.mybir as mybir
from concourse import bacc
import concourse.tile as tile
from concourse import bass_utils

B, N, C, H = 8, 1024, 1024, 16
HD = C // H          # 64
SCALE = HD ** -0.5   # 0.125
P = 128              # SBUF partitions
NT = 512             # moving-dim tile (fp32 PSUM bank limit)
NCH = C // P         # 8 contraction chunks over channels
NMT = N // P         # 8 token tiles of 128
NNT = N // NT        # 2 token tiles of 512
F32 = mybir.dt.float32
F32R = mybir.dt.float32r
EXP = mybir.ActivationFunctionType.Exp


def build_module():
    nc = bacc.Bacc("TRN2", target_bir_lowering=False, debug=False, num_devices=B)

    xT = nc.dram_tensor("xT", [C, N], F32R, kind="ExternalInput").ap()
    wqkT = nc.dram_tensor("wqkT", [C, 2 * C], F32R, kind="ExternalInput").ap()
    wvT = nc.dram_tensor("wvT", [C, C], F32R, kind="ExternalInput").ap()
    wpT = nc.dram_tensor("wpT", [C, C], F32R, kind="ExternalInput").ap()
    bias = nc.dram_tensor("bias_bc", [P, C], F32, kind="ExternalInput").ap()
    ones_col = nc.dram_tensor("ones_col", [P, H], F32R, kind="ExternalInput").ap()
    out = nc.dram_tensor("out", [N, C], F32, kind="ExternalOutput").ap()

    with tile.TileContext(nc) as tc, ExitStack() as ctx:
        dram = ctx.enter_context(tc.tile_pool(name="dram", bufs=1, space="DRAM"))
        qkT_d = dram.tile([2 * C, N], F32R, tag="qkT_d", name="qkT_d")
        rden_d = dram.tile([H * NNT, NT], F32, tag="rden_d", name="rden_d")

        # 8 slots of [128, 1024]: first holds xT, later recycled for expS^T.
        xt_pool = ctx.enter_context(tc.tile_pool(name="xt", bufs=8))
        wqk_pool = ctx.enter_context(tc.tile_pool(name="wqk", bufs=10))
        wst_pool = ctx.enter_context(tc.tile_pool(name="wst", bufs=10))
        stage_pool = ctx.enter_context(tc.tile_pool(name="stage", bufs=4))
        vsb_pool = ctx.enter_context(tc.tile_pool(name="vsb", bufs=1))
        aot_pool = ctx.enter_context(tc.tile_pool(name="aot", bufs=1))
        qkp_pool = ctx.enter_context(tc.tile_pool(name="qkp", bufs=4))
        usb_pool = ctx.enter_context(tc.tile_pool(name="usb", bufs=6))
        den_pool = ctx.enter_context(tc.tile_pool(name="den", bufs=2))
        rbc_pool = ctx.enter_context(tc.tile_pool(name="rbc", bufs=4))
        one_pool = ctx.enter_context(tc.tile_pool(name="one", bufs=1))
        psum = ctx.enter_context(tc.tile_pool(name="psum", bufs=5, space="PSUM"))
        psum_u = ctx.enter_context(tc.tile_pool(name="psum_u", bufs=2, space="PSUM"))

        # ---------- input loads ----------
        xts = []
        for t in range(NCH):
            xt_t = xt_pool.tile([P, N], F32R, tag="xt", name=f"xt{t}")
            nc.sync.dma_start(xt_t, xT[t * P : (t + 1) * P, :])
            xts.append(xt_t)
        bias_sb = one_pool.tile([P, C], F32, tag="bias", name="bias_sb")
        nc.sync.dma_start(bias_sb, bias)

        # ---------- phase 1b: v projection (natural layout + ones cols) ----------
        vsb = []
        for mt in range(NMT):
            v_t = vsb_pool.tile([P, H * (HD + 1)], F32R, tag=f"v{mt}", name=f"v{mt}")
            nc.sync.dma_start(
                v_t.rearrange("p (h w) -> p h w", w=HD + 1)[:, :, HD : HD + 1], ones_col
            )
            vsb.append(v_t)
        for dvt in range(NNT):  # two 512-wide halves of d_v (heads 8*dvt..8*dvt+7)
            wv_tiles = []
            for ck in range(NCH):
                wv_t = wst_pool.tile([P, NT], F32R, tag="wst", name=f"wv{dvt}_{ck}")
                nc.sync.dma_start(
                    wv_t, wvT[ck * P : (ck + 1) * P, dvt * NT : (dvt + 1) * NT]
                )
                wv_tiles.append(wv_t)
            for mt in range(NMT):
                ps_v = psum.tile([P, NT], F32, tag="ps", name=f"psv{mt}_{dvt}")
                for ck in range(NCH):
                    nc.tensor.matmul(
                        ps_v,
                        lhsT=(xts[ck][:, mt * P : (mt + 1) * P]),
                        rhs=(wv_tiles[ck]),
                        start=(ck == 0),
                        stop=(ck == NCH - 1),
                    )
                dst = vsb[mt].rearrange("p (h w) -> p h w", w=HD + 1)[
                    :, dvt * 8 : (dvt + 1) * 8, 0:HD
                ]
                nc.vector.tensor_copy(dst, ps_v.rearrange("p (h w) -> p h w", w=HD))

        # ---------- phase 1a: qk projection -> DRAM bounce ----------
        for dg in range(8):  # 256-wide d-groups over the 2048 qk channels
            wq_tiles = []
            for ck in range(NCH):
                wq_t = wqk_pool.tile([P, 256], F32R, tag="wqk", name=f"wq{dg}_{ck}")
                nc.sync.dma_start(
                    wq_t, wqkT[ck * P : (ck + 1) * P, dg * 256 : (dg + 1) * 256]
                )
                wq_tiles.append(wq_t)
            for ds_ in range(2):
                dt = dg * 2 + ds_
                for nt in range(NNT):
                    ps_qk = psum.tile([P, NT], F32, tag="ps", name=f"psqk{dt}_{nt}")
                    for ck in range(NCH):
                        nc.tensor.matmul(
                            ps_qk,
                            lhsT=(wq_tiles[ck][:, ds_ * P : (ds_ + 1) * P]),
                            rhs=(xts[ck][:, nt * NT : (nt + 1) * NT]),
                            start=(ck == 0),
                            stop=(ck == NCH - 1),
                        )
                    st = stage_pool.tile([P, NT], F32R, tag="stage", name=f"st{dt}_{nt}")
                    nc.vector.tensor_copy(st, ps_qk)
                    nc.sync.dma_start(
                        qkT_d[dt * P : (dt + 1) * P, nt * NT : (nt + 1) * NT], st
                    )

        # ---------- attention-output accumulator (attn_outT[c, n]) ----------
        aot = []
        for t in range(NCH):
            a_t = aot_pool.tile([P, N], F32R, tag=f"aot{t}", name=f"aot{t}")
            aot.append(a_t)

        # ---------- phase 2: attention, per head pair ----------
        for pair in range(H // 2):
            hA = 2 * pair
            qp = qkp_pool.tile([P, N], F32R, tag="qp", name=f"qp{pair}")
            nc.sync.dma_start(qp, qkT_d[hA * HD : hA * HD + P, :])
            kp = qkp_pool.tile([P, N], F32R, tag="kp", name=f"kp{pair}")
            nc.sync.dma_start(kp, qkT_d[C + hA * HD : C + hA * HD + P, :])

            pair_units = []
            for j in range(2):  # head within pair; partitions j*64..j*64+63
                h = hA + j
                pl = slice(j * HD, (j + 1) * HD)
                for nt in range(NNT):
                    exps = [
                        xt_pool.tile([P, N], F32R, tag="xt", name=f"e{h}_{nt}_{q}")
                        for q in range(4)
                    ]
                    for mc in range(NMT):
                        ps_s = psum.tile([P, NT], F32, tag="ps", name=f"pss{h}_{nt}_{mc}")
                        nc.tensor.matmul(
                            ps_s,
                            lhsT=(kp[pl, mc * P : (mc + 1) * P]),
                            rhs=(qp[pl, nt * NT : (nt + 1) * NT]),
                            start=True,
                            stop=True,
                        )
                        nc.scalar.activation(
                            exps[mc // 2][:, (mc % 2) * NT : (mc % 2 + 1) * NT],
                            ps_s,
                            EXP,
                            scale=SCALE,
                        )
                    ps_u = psum_u.tile([HD + 1, NT], F32, tag="pu", name=f"psu{h}_{nt}")
                    for mc in range(NMT):
                        nc.tensor.matmul(
                            ps_u,
                            lhsT=(vsb[mc][:, h * (HD + 1) : (h + 1) * (HD + 1)]),
                            rhs=(exps[mc // 2][:, (mc % 2) * NT : (mc % 2 + 1) * NT]),
                            start=(mc == 0),
                            stop=(mc == NMT - 1),
                        )
                    u_sb = usb_pool.tile([HD + 1, NT], F32, tag="usb", name=f"u{h}_{nt}")
                    nc.vector.tensor_copy(u_sb, ps_u)
                    pair_units.append((h, nt, u_sb))

            # denominators for the 4 (head, nt) units of this pair
            den_g = den_pool.tile([4, NT], F32, tag="den", name=f"den{pair}")
            for i, (h, nt, u_sb) in enumerate(pair_units):
                nc.sync.dma_start(den_g[i : i + 1, :], u_sb[HD : HD + 1, :])
            rden = den_pool.tile([4, NT], F32, tag="rden", name=f"rden{pair}")
            nc.vector.reciprocal_approx_fast(out=rden, in_=den_g)
            nc.sync.dma_start(rden_d[pair * 4 : pair * 4 + 4, :], rden)
            for i, (h, nt, u_sb) in enumerate(pair_units):
                rbc = rbc_pool.tile([HD, NT], F32, tag="rbc", name=f"rbc{h}_{nt}")
                src = rden_d[pair * 4 + i : pair * 4 + i + 1, :]
                bsrc = bass.AP(
                    tensor=src.tensor,
                    offset=src.offset,
                    ap=[[0, HD], list(src.ap[-1])],
                )
                nc.gpsimd.dma_start(out=rbc, in_=bsrc)
                ct, prow = h // 2, (h % 2) * HD
                nc.vector.tensor_mul(
                    aot[ct][prow : prow + HD, nt * NT : (nt + 1) * NT],
                    u_sb[0:HD, :],
                    rbc,
                )

        # ---------- phase 3: output projection + bias ----------
        for dt in range(NNT):
            wp_tiles = []
            for ck in range(NCH):
                wp_t = wst_pool.tile([P, NT], F32R, tag="wst", name=f"wp{dt}_{ck}")
                nc.sync.dma_start(
                    wp_t, wpT[ck * P : (ck + 1) * P, dt * NT : (dt + 1) * NT]
                )
                wp_tiles.append(wp_t)
            for nt2 in range(NMT):
                ps_o = psum.tile([P, NT], F32, tag="ps", name=f"pso{dt}_{nt2}")
                for ck in range(NCH):
                    nc.tensor.matmul(
                        ps_o,
                        lhsT=(aot[ck][:, nt2 * P : (nt2 + 1) * P]),
                        rhs=(wp_tiles[ck]),
                        start=(ck == 0),
                        stop=(ck == NCH - 1),
                    )
                o_sb = stage_pool.tile([P, NT], F32, tag="stage", name=f"o{dt}_{nt2}")
                nc.vector.tensor_add(o_sb, ps_o, bias_sb[:, dt * NT : (dt + 1) * NT])
                nc.sync.dma_start(
                    out[nt2 * P : (nt2 + 1) * P, dt * NT : (dt + 1) * NT], o_sb
                )

    nc.compile()
    return nc


def make_in_maps(x, w_qkv, w_proj, b_proj):
    wqkT = np.ascontiguousarray(w_qkv[: 2 * C].T)
    wvT = np.ascontiguousarray(w_qkv[2 * C :].T)
    wpT = np.ascontiguousarray(w_proj.T)
    bias_bc = np.ascontiguousarray(np.broadcast_to(b_proj, (P, C)))
    ones = np.ones((P, H), dtype=np.float32)
    in_maps = []
    for b in range(B):
        in_maps.append(
            {
                "xT": np.ascontiguousarray(x[b].T),
                "wqkT": wqkT,
                "wvT": wvT,
                "wpT": wpT,
                "bias_bc": bias_bc,
                "ones_col": ones,
            }
        )
    return in_maps


_CACHED_NC = None


def kernel(x, w_qkv, w_proj, b_proj):
    global _CACHED_NC
    x = np.asarray(x, dtype=np.float32)
    w_qkv = np.asarray(w_qkv, dtype=np.float32)
    w_proj = np.asarray(w_proj, dtype=np.float32)
    b_proj = np.asarray(b_proj, dtype=np.float32)
    if _CACHED_NC is None:
        _CACHED_NC = build_module()
    nc = _CACHED_NC
    in_maps = make_in_maps(x, w_qkv, w_proj, b_proj)
    res = bass_utils.run_bass_kernel_spmd(nc, in_maps, core_ids=list(range(B)))
    return np.stack([res.results[b]["out"] for b in range(B)], axis=0)


if __name__ == "__main__":
    nc = build_module()
    ninst = sum(len(b.instructions) for b in nc.m.functions[0].blocks)
    print("module built ok;", ninst, "instructions")
